# revision 1
# baseline (speedup 1.0000x reference)
"""MoE (top-1 routing, capacity-dropped) forward on 8 Trainium2 NeuronCores.

Strategy (expert-parallel, per the sharding hint):
  - Host computes the top-1 gating (softmax over E=8 logits per token), the
    per-expert token ranks (cumsum order), capacity dropping, and dispatches
    token rows to their expert: this *is* the sharding step — tokens are
    sharded along the expert axis E, one expert per NeuronCore, with W1/b1/
    W2/b2 sharded along E and the gate Wg applied once on the (replicated)
    full token set.
  - Each core runs the heavy expert FFN on its C=1024 dispatched tokens in
    transposed [feature, token] layout so both matmuls consume the natural
    weight layouts:  yT = W2[e].T @ relu(W1[e].T @ xT + b1[e]) + b2[e].
    Matmuls run as float32r (full fp32 data, full-rate PE mode).
  - Host combine: scatter each expert's output rows back to their token
    positions scaled by the gate value (zero rows for dropped tokens).

Hardcoded shapes: x [4, 2048, 1024], Wg [1024, 8], W1 [8, 1024, 4096],
b1 [8, 4096], W2 [8, 4096, 1024], b2 [8, 1024].
"""

import os
import numpy as np
from contextlib import ExitStack

# The axon NTFF profiling hook is absent in some client environments; if
# tracing is requested via BASS_TRACE but the hook can't be imported,
# run_bass_kernel_spmd would crash. Disable tracing only in that case.
try:
    from antenv.axon_hooks import get_axon_ntff_profile_hook  # noqa: F401
except Exception:
    os.environ.setdefault("BASS_NEVER_TRACE", "1")

from concourse import bacc, mybir, tile
from concourse.bass_utils import run_bass_kernel_spmd

B, S, H, F, E = 4, 2048, 1024, 4096, 8
T = B * S                  # 8192 tokens
C = -(-T // E)             # 1024 capacity per expert
P = 128                    # SBUF partitions
NB = 512                   # matmul moving free-dim block (one PSUM bank, fp32)
NCORES = 8

_DT = mybir.dt.float32r    # fp32 bits, full-rate PE matmul mode


def _build_expert_ffn(repeat: int = 1):
    """Per-core program: yT = W2.T @ relu(W1.T @ xT + b1) + b2, all [feat, tok].

    repeat>1 wraps the whole body in a hardware For loop — used only by the
    timing harness to measure steady-state per-iteration HW time via the
    wall-clock slope between repeat counts."""
    nc = bacc.Bacc("TRN2", target_bir_lowering=False, debug=False,
                   num_devices=NCORES)
    KC = 8  # k-tiles per streamed W2 chunk (matches the W1 slab size)
    # Weights arrive pre-tiled from the host (see kernel()) so every slab DMA
    # is a fully contiguous 4KB-per-partition read instead of a strided
    # 512B-descriptor gather: W1p[m, p, k, f] = W1[k*P+p, m*P+f] and
    # W2p[mh, kc, p, k, f] = W2[kc*KC*P + k*P + p, mh*P + f].
    xT = nc.declare_dram_parameter("xT", [H, C], _DT, isOutput=False)
    W1 = nc.declare_dram_parameter("W1", [F // P, P, H // P, P], _DT, isOutput=False)
    W2 = nc.declare_dram_parameter(
        "W2", [H // P, F // (KC * P), P, KC, P], _DT, isOutput=False)
    b1 = nc.declare_dram_parameter("b1", [P, F // P], mybir.dt.float32,
                                   isOutput=False)  # host-pretiled [p, m]
    b2 = nc.declare_dram_parameter("b2", [P, H // P], mybir.dt.float32,
                                   isOutput=False)
    out = nc.declare_dram_parameter("out", [H, C], mybir.dt.float32, isOutput=True)

    KH = H // P    # 8  k-tiles over H (mm1 contraction)
    KF = F // P    # 32 k-tiles over F (mm2 contraction)
    MF = F // P    # 32 m-tiles over F (mm1 output partitions)
    MH = H // P    # 8  m-tiles over H (mm2 output partitions)
    NBLK = C // NB  # 2 moving blocks over the C tokens

    with tile.TileContext(nc) as tc, ExitStack() as ctx:
        xpool = ctx.enter_context(tc.tile_pool(name="xpool", bufs=1))
        h1pool = ctx.enter_context(tc.tile_pool(name="h1pool", bufs=1))
        cpool = ctx.enter_context(tc.tile_pool(name="cpool", bufs=1))
        wpool = ctx.enter_context(tc.tile_pool(name="wpool", bufs=6))
        ypool = ctx.enter_context(tc.tile_pool(name="ypool", bufs=2))
        psum = ctx.enter_context(tc.tile_pool(name="psum", bufs=6, space="PSUM"))

        loop_ctx = tc.For_i(0, repeat, 1) if repeat > 1 else None
        if loop_ctx is not None:
            loop_ctx.__enter__()

        # biases arrive host-pretiled: b1[p, m] = b1_orig[m*P + p]
        b1_sb = cpool.tile([P, MF], mybir.dt.float32, name="b1_sb")
        nc.sync.dma_start(out=b1_sb[:], in_=b1[:])
        b2_sb = cpool.tile([P, MH], mybir.dt.float32, name="b2_sb")
        nc.sync.dma_start(out=b2_sb[:], in_=b2[:])

        # resident activations
        x_sb = xpool.tile([P, KH, C], _DT, name="x_sb")
        for k in range(KH):
            nc.sync.dma_start(out=x_sb[:, k, :], in_=xT[k * P:(k + 1) * P, :])
        h1_sb = h1pool.tile([P, KF, C], _DT, name="h1_sb")

        # mm1: h1[m*P+p, c] = relu(b1[m*P+p] + sum_h W1[h, m*P+p] * xT[h, c])
        for m in range(MF):
            w1s = wpool.tile([P, KH, P], _DT, tag="wslab", name="w1s")
            nc.scalar.dma_start(out=w1s[:], in_=W1[m])
            for n in range(NBLK):
                ps = psum.tile([P, NB], mybir.dt.float32, tag="ps", name="ps")
                for k in range(KH):
                    nc.tensor.matmul(
                        out=ps[:],
                        lhsT=w1s[:, k, :],
                        rhs=x_sb[:, k, n * NB:(n + 1) * NB],
                        start=(k == 0),
                        stop=(k == KH - 1),
                    )
                nc.scalar.activation(
                    out=h1_sb[:, m, n * NB:(n + 1) * NB],
                    in_=ps[:],
                    func=mybir.ActivationFunctionType.Relu,
                    bias=b1_sb[:, m:m + 1],
                )

        # mm2: y[mh*P+p, c] = b2[mh*P+p] + sum_f W2[f, mh*P+p] * h1[f, c]
        for mh in range(MH):
            w2chunks = []
            for kc in range(KF // KC):
                w2s = wpool.tile([P, KC, P], _DT, tag="wslab", name="w2s")
                nc.scalar.dma_start(out=w2s[:], in_=W2[mh, kc])
                w2chunks.append(w2s)
            for n in range(NBLK):
                ps2 = psum.tile([P, NB], mybir.dt.float32, tag="ps", name="ps2")
                for k in range(KF):
                    nc.tensor.matmul(
                        out=ps2[:],
                        lhsT=w2chunks[k // KC][:, k % KC, :],
                        rhs=h1_sb[:, k, n * NB:(n + 1) * NB],
                        start=(k == 0),
                        stop=(k == KF - 1),
                    )
                y_sb = ypool.tile([P, NB], mybir.dt.float32, tag="y", name="y_sb")
                nc.vector.tensor_add(
                    out=y_sb[:],
                    in0=ps2[:],
                    in1=b2_sb[:, mh:mh + 1].to_broadcast([P, NB]),
                )
                nc.gpsimd.dma_start(
                    out=out[mh * P:(mh + 1) * P, n * NB:(n + 1) * NB], in_=y_sb[:]
                )
        if loop_ctx is not None:
            loop_ctx.__exit__(None, None, None)
    nc.compile()
    return nc


_NC_CACHE = None


def _get_nc():
    global _NC_CACHE
    if _NC_CACHE is None:
        _NC_CACHE = _build_expert_ffn()
    return _NC_CACHE


def _route(tokens: np.ndarray, Wg: np.ndarray):
    """Top-1 gating with capacity C, matching deepspeed top1gating semantics.

    Runs on jax-CPU mirroring the reference ops 1:1 so the discrete routing
    decisions (argmax expert, cumsum slot order, capacity drops) are
    bit-identical to the jax reference — a numpy BLAS logits matmul could
    flip the argmax for tokens whose top-2 logits are ~1e-6 apart.

    Returns per-expert kept token ids (slot order) and per-token gate values
    (0 for dropped tokens)."""
    import jax
    import jax.numpy as jnp

    cpu = jax.devices("cpu")[0]
    with jax.default_device(cpu):
        tok = jnp.asarray(tokens)
        logits = tok @ jnp.asarray(Wg)                       # [T, E]
        gates = jax.nn.softmax(logits, axis=-1)
        idx = jnp.argmax(gates, axis=1)
        mask1 = jax.nn.one_hot(idx, E, dtype=gates.dtype)
        locations1 = jnp.cumsum(mask1, axis=0) - 1.0
        mask1 = mask1 * (locations1 < C).astype(gates.dtype)
        gates1 = jnp.sum(gates * mask1, axis=1)              # 0 if dropped

        mask_np = np.asarray(mask1)
        gate_val = np.asarray(gates1, dtype=np.float32)

    tok_ids = [np.nonzero(mask_np[:, e] > 0)[0] for e in range(E)]
    return tok_ids, gate_val


def kernel(x, Wg, W1, b1, W2, b2):
    x = np.asarray(x, dtype=np.float32)
    Wg = np.asarray(Wg, dtype=np.float32)
    W1 = np.asarray(W1, dtype=np.float32)
    b1 = np.asarray(b1, dtype=np.float32)
    W2 = np.asarray(W2, dtype=np.float32)
    b2 = np.asarray(b2, dtype=np.float32)

    tokens = x.reshape(T, H)
    tok_ids, gate_val = _route(tokens, Wg)

    # dispatch: shard tokens along the expert axis, one expert per core.
    # Weights are pre-tiled to the kernel's SBUF slab layout (see
    # _build_expert_ffn) so each slab DMA is contiguous per partition.
    KC = 8
    in_maps = []
    for e in range(E):
        xT_e = np.zeros((H, C), dtype=np.float32)
        ids = tok_ids[e]
        xT_e[:, :len(ids)] = tokens[ids].T
        W1p = np.ascontiguousarray(
            W1[e].reshape(H // P, P, F // P, P).transpose(2, 1, 0, 3))
        W2p = np.ascontiguousarray(
            W2[e].reshape(F // (KC * P), KC, P, H // P, P).transpose(3, 0, 2, 1, 4))
        in_maps.append({
            "xT": xT_e,
            "W1": W1p,
            "W2": W2p,
            "b1": np.ascontiguousarray(b1[e].reshape(F // P, P).T),
            "b2": np.ascontiguousarray(b2[e].reshape(H // P, P).T),
        })

    nc = _get_nc()
    res = run_bass_kernel_spmd(nc, in_maps, list(range(NCORES)))

    # combine: scatter expert outputs back, scaled by the gate value
    out = np.zeros((T, H), dtype=np.float32)
    for e in range(E):
        ids = tok_ids[e]
        yT = res.results[e]["out"]                   # [H, C]
        out[ids] = yT[:, :len(ids)].T * gate_val[ids, None]
    return out.reshape(B, S, H)



# revision 2
# speedup vs baseline: 1.0277x; 1.0277x over previous
"""MoE (top-1 routing, capacity-dropped) forward on 8 Trainium2 NeuronCores.

Strategy (expert-parallel, per the sharding hint):
  - Host computes the top-1 gating (softmax over E=8 logits per token), the
    per-expert token ranks (cumsum order), capacity dropping, and dispatches
    token rows to their expert: tokens are sharded along the expert axis E,
    one expert per NeuronCore, with W1/b1/W2/b2 sharded along E and the gate
    Wg applied once on the full token set.
  - Each core runs the heavy expert FFN on its C=1024 dispatched tokens in
    transposed [feature, token] layout:
        yT = W2[e].T @ relu(W1[e].T @ xT + b1[e]) + b2[e]
    Matmuls run in bfloat16 with fp32 PSUM accumulation (end-to-end rel_fro
    ~3.7e-3, well under the 2e-2 gate). bf16 halves every DMA transfer vs
    fp32r at the same PE streaming rate (measured: the PE is rate-bound at
    ~0.55 ns/column sustained regardless of dtype/weight-load structure).
  - Schedule: x k-tiles are DMAed first, alternating the sync/gpsimd queues
    (b1/b2 demoted behind them) and the first W1 slab is split in half so
    the first matmul chain starts ~4us earlier; matmul chains run k-outer so
    each loaded stationary tile feeds both 512-col moving blocks; the final
    output block shrinks to a 128-col chain so the drain tail is short.
  - Host combine: scatter each expert's output rows back to their token
    positions scaled by the gate value (zero rows for dropped tokens).

Hardcoded shapes: x [4, 2048, 1024], Wg [1024, 8], W1 [8, 1024, 4096],
b1 [8, 4096], W2 [8, 4096, 1024], b2 [8, 1024].
"""

import os
import numpy as np
import ml_dtypes
from contextlib import ExitStack

# The axon NTFF profiling hook is absent in some client environments; if
# tracing is requested via BASS_TRACE but the hook can't be imported,
# run_bass_kernel_spmd would crash. Disable tracing only in that case.
try:
    from antenv.axon_hooks import get_axon_ntff_profile_hook  # noqa: F401
except Exception:
    os.environ.setdefault("BASS_NEVER_TRACE", "1")

from concourse import bacc, mybir, tile
from concourse.bass_utils import run_bass_kernel_spmd

B, S, H, F, E = 4, 2048, 1024, 4096, 8
T = B * S                  # 8192 tokens
C = -(-T // E)             # 1024 capacity per expert
P = 128                    # SBUF partitions
NB = 512                   # matmul moving free-dim block (one PSUM bank, fp32)
NCORES = 8
KC = 8                     # k-tiles per streamed W2 chunk

_DT = mybir.dt.bfloat16
_NPDT = ml_dtypes.bfloat16


def _build_expert_ffn(repeat: int = 1):
    """Per-core program: yT = W2.T @ relu(W1.T @ xT + b1) + b2, all [feat, tok].

    repeat>1 wraps the whole body in a hardware For loop — used only by the
    timing harness to measure steady-state per-iteration HW time via the
    wall-clock slope between repeat counts."""
    nc = bacc.Bacc("TRN2", target_bir_lowering=False, debug=False,
                   num_devices=NCORES)
    # Weights arrive pre-tiled from the host (see kernel()) so every slab DMA
    # is a contiguous read: W1p[m, p, k, f] = W1[k*P+p, m*P+f] and
    # W2p[mh, kc, p, k, f] = W2[kc*KC*P + k*P + p, mh*P + f].
    xT = nc.declare_dram_parameter("xT", [H, C], _DT, isOutput=False)
    W1 = nc.declare_dram_parameter("W1", [F // P, P, H // P, P], _DT, isOutput=False)
    W2 = nc.declare_dram_parameter(
        "W2", [H // P, F // (KC * P), P, KC, P], _DT, isOutput=False)
    b1 = nc.declare_dram_parameter("b1", [P, F // P], mybir.dt.float32,
                                   isOutput=False)  # host-pretiled [p, m]
    b2 = nc.declare_dram_parameter("b2", [P, H // P], mybir.dt.float32,
                                   isOutput=False)
    out = nc.declare_dram_parameter("out", [H, C], _DT, isOutput=True)

    KH = H // P    # 8  k-tiles over H (mm1 contraction)
    KF = F // P    # 32 k-tiles over F (mm2 contraction)
    MF = F // P    # 32 m-tiles over F (mm1 output partitions)
    MH = H // P    # 8  m-tiles over H (mm2 output partitions)
    NBLK = C // NB  # 2 moving blocks over the C tokens

    with tile.TileContext(nc) as tc, ExitStack() as ctx:
        xpool = ctx.enter_context(tc.tile_pool(name="xpool", bufs=1))
        h1pool = ctx.enter_context(tc.tile_pool(name="h1pool", bufs=1))
        cpool = ctx.enter_context(tc.tile_pool(name="cpool", bufs=1))
        wpool = ctx.enter_context(tc.tile_pool(name="wpool", bufs=6))
        ypool = ctx.enter_context(tc.tile_pool(name="ypool", bufs=4))
        psum = ctx.enter_context(tc.tile_pool(name="psum", bufs=6, space="PSUM"))

        loop_ctx = tc.For_i(0, repeat, 1) if repeat > 1 else None
        if loop_ctx is not None:
            loop_ctx.__enter__()

        # head: x k-tiles first, alternating HW DMA queues; biases after.
        x_sb = xpool.tile([P, KH, C], _DT, name="x_sb")
        for k in range(KH):
            q = nc.sync if k % 2 == 0 else nc.gpsimd
            q.dma_start(out=x_sb[:, k, :], in_=xT[k * P:(k + 1) * P, :])
        b1_sb = cpool.tile([P, MF], mybir.dt.float32, name="b1_sb")
        nc.gpsimd.dma_start(out=b1_sb[:], in_=b1[:])
        b2_sb = cpool.tile([P, MH], mybir.dt.float32, name="b2_sb")
        nc.gpsimd.dma_start(out=b2_sb[:], in_=b2[:])

        h1_sb = h1pool.tile([P, KF, C], _DT, name="h1_sb")

        # mm1: h1[m*P+p, c] = relu(b1[m*P+p] + sum_h W1[h, m*P+p] * xT[h, c])
        # k-outer chains: each stationary tile feeds both 512-col blocks.
        for m in range(MF):
            if m == 0:
                # first slab in halves so the first chain starts sooner
                w1a = wpool.tile([P, KH // 2, P], _DT, tag="wslab0", name="w1a")
                nc.scalar.dma_start(out=w1a[:], in_=W1[0][:, 0:KH // 2, :])
                w1b = wpool.tile([P, KH // 2, P], _DT, tag="wslab0", name="w1b")
                nc.scalar.dma_start(out=w1b[:], in_=W1[0][:, KH // 2:KH, :])

                def wtile(k):
                    return w1a[:, k, :] if k < KH // 2 else w1b[:, k - KH // 2, :]
            else:
                w1s = wpool.tile([P, KH, P], _DT, tag="wslab", name="w1s")
                nc.scalar.dma_start(out=w1s[:], in_=W1[m])

                def wtile(k, _w=w1s):
                    return _w[:, k, :]

            pss = [psum.tile([P, NB], mybir.dt.float32, tag="ps", name="ps")
                   for _ in range(NBLK)]
            for k in range(KH):
                for n in range(NBLK):
                    nc.tensor.matmul(
                        out=pss[n][:],
                        lhsT=wtile(k),
                        rhs=x_sb[:, k, n * NB:(n + 1) * NB],
                        start=(k == 0),
                        stop=(k == KH - 1),
                    )
            for n in range(NBLK):
                nc.scalar.activation(
                    out=h1_sb[:, m, n * NB:(n + 1) * NB],
                    in_=pss[n][:],
                    func=mybir.ActivationFunctionType.Relu,
                    bias=b1_sb[:, m:m + 1],
                )

        # mm2: y[mh*P+p, c] = b2[mh*P+p] + sum_f W2[f, mh*P+p] * h1[f, c]
        for mh in range(MH):
            w2chunks = []
            for kc in range(KF // KC):
                w2s = wpool.tile([P, KC, P], _DT, tag="wslab", name="w2s")
                nc.scalar.dma_start(out=w2s[:], in_=W2[mh, kc])
                w2chunks.append(w2s)

            def w2tile(k):
                return w2chunks[k // KC][:, k % KC, :]

            if mh < MH - 1:
                pss = [psum.tile([P, NB], mybir.dt.float32, tag="ps", name="ps2")
                       for _ in range(NBLK)]
                for k in range(KF):
                    for n in range(NBLK):
                        nc.tensor.matmul(
                            out=pss[n][:],
                            lhsT=w2tile(k),
                            rhs=h1_sb[:, k, n * NB:(n + 1) * NB],
                            start=(k == 0),
                            stop=(k == KF - 1),
                        )
                for n in range(NBLK):
                    y_sb = ypool.tile([P, NB], _DT, tag="y", name="y_sb")
                    nc.vector.tensor_add(
                        out=y_sb[:],
                        in0=pss[n][:],
                        in1=b2_sb[:, mh:mh + 1].to_broadcast([P, NB]),
                    )
                    nc.gpsimd.dma_start(
                        out=out[mh * P:(mh + 1) * P, n * NB:(n + 1) * NB],
                        in_=y_sb[:])
            else:
                # last row of output tiles: shrinking blocks so only a
                # 128-col chain drains at the very end of the program.
                lo = 0
                for nb_i in (512, 384, 128):
                    ps2 = psum.tile([P, nb_i], mybir.dt.float32, tag="ps",
                                    name="ps2")
                    for k in range(KF):
                        nc.tensor.matmul(
                            out=ps2[:],
                            lhsT=w2tile(k),
                            rhs=h1_sb[:, k, lo:lo + nb_i],
                            start=(k == 0),
                            stop=(k == KF - 1),
                        )
                    y_sb = ypool.tile([P, nb_i], _DT, tag="y", name="y_sb")
                    nc.vector.tensor_add(
                        out=y_sb[:],
                        in0=ps2[:],
                        in1=b2_sb[:, mh:mh + 1].to_broadcast([P, nb_i]),
                    )
                    q = nc.sync if nb_i == 128 else nc.gpsimd
                    q.dma_start(
                        out=out[mh * P:(mh + 1) * P, lo:lo + nb_i],
                        in_=y_sb[:])
                    lo += nb_i
        if loop_ctx is not None:
            loop_ctx.__exit__(None, None, None)
    nc.compile()
    return nc


_NC_CACHE = None


def _get_nc():
    global _NC_CACHE
    if _NC_CACHE is None:
        _NC_CACHE = _build_expert_ffn()
    return _NC_CACHE


def _route(tokens: np.ndarray, Wg: np.ndarray):
    """Top-1 gating with capacity C, matching deepspeed top1gating semantics.

    Runs on jax-CPU mirroring the reference ops 1:1 so the discrete routing
    decisions (argmax expert, cumsum slot order, capacity drops) are
    bit-identical to the jax reference — a numpy BLAS logits matmul could
    flip the argmax for tokens whose top-2 logits are ~1e-6 apart.

    Returns per-expert kept token ids (slot order) and per-token gate values
    (0 for dropped tokens)."""
    import jax
    import jax.numpy as jnp

    cpu = jax.devices("cpu")[0]
    with jax.default_device(cpu):
        tok = jnp.asarray(tokens)
        logits = tok @ jnp.asarray(Wg)                       # [T, E]
        gates = jax.nn.softmax(logits, axis=-1)
        idx = jnp.argmax(gates, axis=1)
        mask1 = jax.nn.one_hot(idx, E, dtype=gates.dtype)
        locations1 = jnp.cumsum(mask1, axis=0) - 1.0
        mask1 = mask1 * (locations1 < C).astype(gates.dtype)
        gates1 = jnp.sum(gates * mask1, axis=1)              # 0 if dropped

        mask_np = np.asarray(mask1)
        gate_val = np.asarray(gates1, dtype=np.float32)

    tok_ids = [np.nonzero(mask_np[:, e] > 0)[0] for e in range(E)]
    return tok_ids, gate_val


def _prepare_in_maps(x, Wg, W1, b1, W2, b2):
    """Route on host, then build per-core (per-expert) input dicts in the
    kernel's pretiled bf16 layouts. Returns (in_maps, tok_ids, gate_val)."""
    tokens = x.reshape(T, H)
    tok_ids, gate_val = _route(tokens, Wg)

    in_maps = []
    for e in range(E):
        xT_e = np.zeros((H, C), dtype=_NPDT)
        ids = tok_ids[e]
        xT_e[:, :len(ids)] = tokens[ids].astype(_NPDT).T
        W1p = np.ascontiguousarray(
            W1[e].astype(_NPDT).reshape(H // P, P, F // P, P).transpose(2, 1, 0, 3))
        W2p = np.ascontiguousarray(
            W2[e].astype(_NPDT).reshape(F // (KC * P), KC, P, H // P, P)
            .transpose(3, 0, 2, 1, 4))
        in_maps.append({
            "xT": xT_e,
            "W1": W1p,
            "W2": W2p,
            "b1": np.ascontiguousarray(b1[e].reshape(F // P, P).T),
            "b2": np.ascontiguousarray(b2[e].reshape(H // P, P).T),
        })
    return in_maps, tok_ids, gate_val


def kernel(x, Wg, W1, b1, W2, b2):
    x = np.asarray(x, dtype=np.float32)
    Wg = np.asarray(Wg, dtype=np.float32)
    W1 = np.asarray(W1, dtype=np.float32)
    b1 = np.asarray(b1, dtype=np.float32)
    W2 = np.asarray(W2, dtype=np.float32)
    b2 = np.asarray(b2, dtype=np.float32)

    in_maps, tok_ids, gate_val = _prepare_in_maps(x, Wg, W1, b1, W2, b2)

    nc = _get_nc()
    res = run_bass_kernel_spmd(nc, in_maps, list(range(NCORES)))

    # combine: scatter expert outputs back, scaled by the gate value
    out = np.zeros((T, H), dtype=np.float32)
    for e in range(E):
        ids = tok_ids[e]
        yT = np.asarray(res.results[e]["out"]).astype(np.float32)   # [H, C]
        out[ids] = yT[:, :len(ids)].T * gate_val[ids, None]
    return out.reshape(B, S, H)
